# revision 23
# baseline (speedup 1.0000x reference)
"""Trainium2 Bass kernel for nn_CrossAttention (B=4, C=512, H=W=64, CQK=64).

Math (per batch b):
    Q = Wq @ rgb + bq                      [CQK, HW]
    K = Wk @ chm + bk                      [CQK, XY]
    S[hw, xy] = sum_o Q[o, hw] K[o, xy]
    P = softmax over y only (xy = x*64 + y)
    att[c, hw] = sum_xy P[hw, xy] (Wv @ chm + bv)[c, xy]
    out = rgb + gamma * att
Sharding: 8 cores = 4 batches x 2 halves of the hw (query) axis.

Key ideas (on top of the bf16 transposed-scores pipeline):
  - The attend GEMM (chm @ P^T, 8.6 GFLOP/core - 2/3 of all PE work) runs in
    fp8e4 with perf_mode=DoubleRow: each matmul contracts TWO 128-row xy
    tiles at half the per-row cost, a 4x reduction of the dominant GEMM's PE
    time. rel-err stays ~1.6e-2 (< 2e-2): P is in [0,1] post-normalize and
    chm ~ N(0,1), both well inside e4m3 range.
  - chm^T arrives PRE-TRANSPOSED in fp8 from the host (2MB), replacing the
    4MB on-device xbar transposes entirely.
  - Softmax normalize stays bf16 on DVE (2x mode); the bf16->fp8 conversion
    of P~ rides the (otherwise idle) DMA engines as gpsimd cast-DMAs, so DVE
    never pays the 1x-mode fp8-output penalty.
  - Scores, Q/K/V GEMMs and M2 (Wv apply) stay bf16: fp8 there fails the
    error budget (measured in a numpy prototype).
  - Engine split: PE matmuls only; ACT exps + qt bias; DVE tree-sums,
    reciprocal, normalize; Pool (gpsimd) runs the bulk-load + cast-DMA ring
    AND takes the M1 PSUM->SBUF copies and final rgb adds, pulling both off
    the busier ACT/DVE.
  - gamma and bv fold on the host (bv contributes 64*gamma*bv[c] since
    softmax rows sum to 1 per (hw, x) and there are 64 x's).
"""

from contextlib import ExitStack

import numpy as np

import concourse.bass as bass
import concourse.mybir as mybir
import concourse.tile as tile
from concourse import bacc
from concourse.bass_utils import run_bass_kernel_spmd

P = 128
B, C, H, W = 4, 512, 64, 64
HW = H * W                # 4096
CQK = C // 8              # 64
N_CORES = 8
HWC = HW // 2             # hw rows per core (2048)
XY = HW                   # key/value positions per batch (4096)

F32 = mybir.dt.float32
BF16 = mybir.dt.bfloat16
FP8 = mybir.dt.float8e4
ADD = mybir.AluOpType.add
MULT = mybir.AluOpType.mult
IDENT = mybir.ActivationFunctionType.Identity
EXP = mybir.ActivationFunctionType.Exp
DROW = mybir.MatmulPerfMode.DoubleRow


def build_program(hwc=HWC, xy=XY, c=C, cqk=CQK, n_cores=N_CORES):
    """Build the per-core Bass program. Returns a compiled Bacc module."""
    ck = c // P               # channel chunks (4)
    nb = hwc // 512           # hw blocks (4)
    xt = xy // P              # xy tiles (32)
    xb = xy // 512            # xy 512-blocks (8)

    nc = bacc.Bacc("TRN2", target_bir_lowering=False, debug=False,
                   num_devices=n_cores)
    ld = nc.gpsimd          # bulk loads + cast-DMAs + ALU offload
    st = nc.sync            # output stores
    wl = nc.scalar          # small weight loads (own HWDGE ring)

    rgb = nc.dram_tensor("rgb", [c, hwc], BF16, kind="ExternalInput")
    chm = nc.dram_tensor("chm", [c, xy], BF16, kind="ExternalInput")
    chmT8 = nc.dram_tensor("chmT8", [P, xt * ck * P], FP8, kind="ExternalInput")
    wqk = nc.dram_tensor("wqk", [c, 4 * cqk], BF16, kind="ExternalInput")
    wv = nc.dram_tensor("wv", [c, c], BF16, kind="ExternalInput")
    bs = nc.dram_tensor("bs", [2 * cqk, 2], F32, kind="ExternalInput")
    comb = nc.dram_tensor("comb", [P, P], BF16, kind="ExternalInput")
    out = nc.dram_tensor("out", [c, hwc], F32, kind="ExternalOutput")

    rgb_t = rgb.ap().rearrange("(k p) n -> p k n", p=P)
    chm_t = chm.ap().rearrange("(k p) n -> p k n", p=P)
    chmT8_t = chmT8.ap().rearrange("p (m k q) -> p m k q", m=xt, k=ck)
    wqk_t = wqk.ap().rearrange("(k p) m -> p k m", p=P)
    wv_t = wv.ap().rearrange("(k p) m -> p k m", p=P)
    out_t = out.ap().rearrange("(k p) n -> p k n", p=P)

    with tile.TileContext(nc) as tc:
        with tc.tile_pool(name="pers", bufs=1) as pers, \
             tc.tile_pool(name="ptpool", bufs=2) as ptpool, \
             tc.tile_pool(name="pt8pool", bufs=2) as pt8pool, \
             nc.allow_low_precision(reason="softmax weights in fp8/bf16"):
            # --- persistent tiles ---
            wq_sb = pers.tile([P, ck, 2 * cqk], BF16)
            wk_sb = pers.tile([P, ck, 2 * cqk], BF16)
            wv_sb = pers.tile([P, ck, c], BF16)
            comb_sb = pers.tile([P, P], BF16)
            bq_sb = pers.tile([2 * cqk, 1], F32)
            bk_sb = pers.tile([2 * cqk, 1], F32)
            qt_t = [pers.tile([2 * cqk, 512], BF16, name=f"qt{j}")
                    for j in range(nb)]
            kf_t = [pers.tile([2 * cqk, 512], BF16, name=f"kf{j}")
                    for j in range(xb)]
            # chm^T in fp8, loaded pre-transposed from the host.
            chmT8_sb = pers.tile([P, xt, ck, P], FP8)

            rgb_sb = [pers.tile([P, hwc], BF16, name=f"rgb{k}")
                      for k in range(ck)]
            stk = ExitStack()
            SBS = [(0, 0, 512), (1, 0, 512),
                   (2, 0, 512), (3, 0, 512)]
            NS = len(SBS)
            ptb = {}
            pt8 = {}
            scr = {}
            rz = {}

            def score_pair(s, t):
                """One pair of packed score matmuls + exp for sub-block s.
                P~^T lives in EIGHT separate 4-m half-tiles so tile-granular
                dependency tracking lets tree quarters, norm pieces, casts
                and attend m-segments wait only on writes they consume."""
                j, off, w = SBS[s]
                if t == 0:
                    ptb[s] = [ptpool.tile([P, 8, 512], BF16, tag=f"pt{i}",
                                          name=f"ptb{s}_{i}")
                              for i in range(4)]
                    pt8[s] = [pt8pool.tile([P, 8, 512], FP8, tag=f"p8{i}",
                                           name=f"pt8{s}_{i}")
                              for i in range(4)]
                s_ps = psS.tile([P, 1024], F32, tag="sps")
                sv = s_ps[:].rearrange("p (a n) -> p a n", a=2)
                m0, m1_ = 2 * t, 2 * t + 1
                nc.tensor.matmul(
                    sv[:, 0, 0:w],
                    kf_t[m0 // 4][0:cqk, P * (m0 % 4):P * (m0 % 4 + 1)],
                    qt_t[j][0:cqk, off:off + w],
                    start=True, stop=True, tile_position=(0, 0))
                nc.tensor.matmul(
                    sv[:, 1, 0:w],
                    kf_t[m1_ // 4][cqk:2 * cqk,
                                   P * (m1_ % 4):P * (m1_ % 4 + 1)],
                    qt_t[j][cqk:2 * cqk, off:off + w],
                    start=True, stop=True, tile_position=(cqk, 0))
                nc.scalar.activation(
                    ptb[s][t // 4][:, 2 * (t % 4):2 * (t % 4) + 2, 0:w],
                    sv[:, :, 0:w], EXP)

            def tree_q(s, i):
                """Quarter y-sum into two half-scratch tiles; the A/B
                half-folds run as soon as their two quarters exist. The
                even quarters run on the (otherwise idle) Pool engine —
                SBUF-only ops are GPSIMD-legal — pulling ~2.3us/sub-block
                off the busier DVE."""
                w = SBS[s][2]
                if i == 0:
                    scr[s] = [scrp.tile([P, 8, 512], BF16, tag=f"scr{h}",
                                        name=f"scr{s}_{h}")
                              for h in range(2)]
                sc = scr[s][i // 2]
                lo = 4 * (i % 2)
                eng = ld if i % 2 == 0 else nc.vector
                eng.tensor_tensor(
                    sc[:, lo:lo + 4, 0:w], ptb[s][i][:, 0:4, 0:w],
                    ptb[s][i][:, 4:8, 0:w], ADD)
                if i % 2 == 1:
                    nc.vector.tensor_tensor(sc[:, 0:4, 0:w],
                                            sc[:, 0:4, 0:w],
                                            sc[:, 4:8, 0:w], ADD)

            def tree_folds(s):
                """Final folds down to z1 = scr[s][0][:, 0]."""
                w = SBS[s][2]
                sa, sb_ = scr[s]
                nc.vector.tensor_tensor(sa[:, 0:4, 0:w], sa[:, 0:4, 0:w],
                                        sb_[:, 0:4, 0:w], ADD)
                nc.vector.tensor_tensor(sa[:, 0:2, 0:w], sa[:, 0:2, 0:w],
                                        sa[:, 2:4, 0:w], ADD)
                nc.vector.tensor_tensor(sa[:, 0:1, 0:w], sa[:, 0:1, 0:w],
                                        sa[:, 1:2, 0:w], ADD)

            def zc_recip(s):
                """Combine partition y-halves on PE, then reciprocal.
                Z borrows a psS slot: by emission time all of this
                sub-block's score pairs are already out, so the only WAR
                chaining is the next sub-block's first pair against the
                (prompt) reciprocal read."""
                w = SBS[s][2]
                z_ps = psS.tile([P, 1024], F32, tag="sps", name=f"zps{s}")
                nc.tensor.matmul(z_ps[:, 0:w], comb_sb[:],
                                 scr[s][0][:, 0, 0:w],
                                 start=True, stop=True)
                r = rzp.tile([P, 1, 512], BF16, tag="rz", name=f"rz{s}")
                rz[s] = r
                nc.vector.reciprocal(r[:, :, 0:w], z_ps[:, 0:w])

            def norm_one(s):
                """P~ *= 1/Z in bf16 (DVE 2x mode), then a gpsimd cast-DMA
                converts each 4-m half-tile to fp8 on the DMA engines. The
                fine granularity keeps the normalize->cast->attend latency
                per piece low, so attend starts ~2.5us after the first
                normalize instead of waiting for a full-tile cast."""
                w = SBS[s][2]
                for i in range(4):
                    nc.vector.tensor_tensor(
                        ptb[s][i][:, :, 0:w], ptb[s][i][:, :, 0:w],
                        rz[s][:, :, 0:w].to_broadcast([P, 8, w]), MULT)
                    ld.dma_start(pt8[s][i][:, :, 0:w], ptb[s][i][:, :, 0:w])

            def attend_m1(s, m_ps, weave_pairs, mid_cb=None, end_cb=None):
                """M1 = chm8 @ P8^T as fp8 DoubleRow pair-matmuls, with the
                next-next sub-block's score pairs woven between them so the
                ACT exp stream stays fed. mid_cb (the next sub-block's Z
                pipeline) fires near the window START: its exps finished a
                full window ago, so the casts land well before its attend
                begins. end_cb fires after the last M1 matmul (used when
                the next Z's tree inputs are only just arriving — a PE z
                matmul earlier in the window would stall the attend)."""
                w = SBS[s][2]
                pi = 0
                for mp in range(xt // 2):
                    if pi < len(weave_pairs):
                        ns_, t = weave_pairs[pi]
                        score_pair(ns_, t)
                        if t % 4 == 3:
                            tree_q(ns_, t // 4)
                        pi += 1
                    if mp == 3 and mid_cb is not None:
                        mid_cb()
                    ti, sl = mp // 4, 2 * (mp % 4)
                    for ch in range(ck):
                        nc.tensor.matmul(
                            m_ps[ch][:, 0:w],
                            chmT8_sb[:, 2 * mp:2 * mp + 2, ch, :],
                            pt8[s][ti][:, sl:sl + 2, 0:w],
                            start=(mp == 0), stop=(mp == xt // 2 - 1),
                            perf_mode=DROW)
                if end_cb is not None:
                    end_cb()

            def att2_store(s, m1_sb):
                """M2 = (gamma Wv) @ M1, rgb add on Pool, store. ct-outer
                so a_ps[0] completes (and its Pool add frees the psA slot)
                before the next sub-block's attend needs it."""
                j, off, w = SBS[s]
                o_sb = op.tile([P, ck, 512], F32, tag="o", name=f"o{s}")
                a_ps = [psA.tile([P, 512], F32, tag="aps",
                                 name=f"aps{s}_{ct}") for ct in range(ck)]
                if s == NS - 1:
                    # tail: ch-outer means each ACT copy unblocks 4 matmuls
                    # instead of one, so M2 ends ~1us after the last copy.
                    for ch in range(ck):
                        for ct in range(ck):
                            nc.tensor.matmul(
                                a_ps[ct][:, 0:w],
                                wv_sb[:, ch, P * ct:P * (ct + 1)],
                                m1_sb[:, ch, 0:w],
                                start=(ch == 0), stop=(ch == ck - 1))
                    for ct in range(ck):
                        nc.vector.tensor_tensor(
                            o_sb[:, ct, 0:w], a_ps[ct][:, 0:w],
                            rgb_sb[ct][:, 512 * j + off:512 * j + off + w],
                            ADD)
                        nc.sync.dma_start(
                            out_t[:, ct:ct + 1,
                                  512 * j + off:512 * j + off + w],
                            o_sb[:, ct:ct + 1, 0:w])
                    return
                for ct in range(ck):
                    for ch in range(ck):
                        nc.tensor.matmul(
                            a_ps[ct][:, 0:w], wv_sb[:, ch, P * ct:P * (ct + 1)],
                            m1_sb[:, ch, 0:w],
                            start=(ch == 0), stop=(ch == ck - 1))
                    nc.vector.tensor_tensor(
                        o_sb[:, ct, 0:w], a_ps[ct][:, 0:w],
                        rgb_sb[ct][:, 512 * j + off:512 * j + off + w],
                        ADD)
                    nc.sync.dma_start(
                        out_t[:, ct:ct + 1,
                              512 * j + off:512 * j + off + w],
                        o_sb[:, ct:ct + 1, 0:w])

            with tc.tile_pool(name="chmp", bufs=1) as chmp:
                # merged-load landing tiles live in this scoped pool
                wqk_sb = chmp.tile([P, ck, 4 * cqk], BF16)
                bs_sb = chmp.tile([2 * cqk, 2], F32)
                wl.dma_start(wqk_sb[:], wqk_t)
                wl.dma_start(bs_sb[:], bs.ap())
                wl.dma_start(comb_sb[:], comb.ap())
                nc.vector.tensor_copy(wq_sb[:], wqk_sb[:, :, 0:2 * cqk])
                nc.vector.tensor_copy(wk_sb[:], wqk_sb[:, :, 2 * cqk:4 * cqk])
                nc.vector.tensor_copy(bq_sb[:], bs_sb[:, 0:1])
                nc.vector.tensor_copy(bk_sb[:], bs_sb[:, 1:2])
                # split the bulk loads across the gpsimd AND sync DMA rings
                # (stores only start ~40us later), chm chunks first: the K
                # GEMM (chm-gated) feeds the score+exp backbone, which paces
                # the whole kernel.
                chm_sb = [chmp.tile([P, xy], BF16, name=f"chm{k}")
                          for k in range(ck)]
                for k in range(ck):
                    ld.dma_start(rgb_sb[k][:], rgb_t[:, k])
                for k in range(ck):
                    ld.dma_start(chm_sb[k][:], chm_t[:, k])
                # --- Q GEMM first: qt[o, hw]; bias-add on ACT ---
                with tc.tile_pool(name="psQ", bufs=1, space="PSUM") as psQ:
                    q_ps = [psQ.tile([2 * cqk, 512], F32, name=f"qps{i}")
                            for i in range(nb)]
                    for k in range(ck):
                        for j in range(nb):
                            nc.tensor.matmul(
                                q_ps[j][:], wq_sb[:, k],
                                rgb_sb[k][:, 512 * j:512 * (j + 1)],
                                start=(k == 0), stop=(k == ck - 1))
                    for j in range(nb):
                        nc.scalar.activation(qt_t[j][:], q_ps[j][:], IDENT,
                                             bias=bq_sb[:])

                # --- K GEMM: kf[o, xy']; bias-adds split DVE/ACT ---
                with tc.tile_wait_until(0.036):
                    wl.dma_start(wv_sb[:], wv_t)
                    wl.dma_start(chmT8_sb[:], chmT8_t)
                # K GEMM runs in TWO 4-bank passes: pass 1 (kf j0-3)
                # closes early so psS can open and the first score pairs
                # overlap pass 2 (kf j4-7) on the other four banks.
                with tc.tile_pool(name="psK1", bufs=1, space="PSUM") as psK1:
                    k_ps = [psK1.tile([2 * cqk, 512], F32, name=f"kps{i}")
                            for i in range(4)]
                    for k in range(ck):
                        for j in range(4):
                            nc.tensor.matmul(
                                k_ps[j][:], wk_sb[:, k],
                                chm_sb[k][:, 512 * j:512 * (j + 1)],
                                start=(k == 0), stop=(k == ck - 1))
                    for j in range(4):
                        if j % 2 == 0:
                            nc.vector.tensor_scalar_add(kf_t[j][:],
                                                        k_ps[j][:], bk_sb[:])
                        else:
                            nc.scalar.activation(kf_t[j][:], k_ps[j][:],
                                                 IDENT, bias=bk_sb[:])

                # open psS while chm is still resident: the first score
                # pairs (kf j0-3) overlap K GEMM pass 2 (kf j4-7)
                psS = stk.enter_context(
                    tc.tile_pool(name="psS", bufs=2, space="PSUM"))
                with tc.tile_pool(name="psK2", bufs=1, space="PSUM") as psK2:
                    k_ps2 = [psK2.tile([2 * cqk, 512], F32, name=f"kps2{i}")
                             for i in range(4)]
                    for k in range(ck):
                        for j in range(4):
                            nc.tensor.matmul(
                                k_ps2[j][:], wk_sb[:, k],
                                chm_sb[k][:, 2048 + 512 * j:
                                            2048 + 512 * (j + 1)],
                                start=(k == 0), stop=(k == ck - 1))
                    for t in range(8):
                        score_pair(0, t)
                    for j in range(4):
                        if j % 2 == 0:
                            nc.vector.tensor_scalar_add(
                                kf_t[4 + j][:], k_ps2[j][:], bk_sb[:])
                        else:
                            nc.scalar.activation(kf_t[4 + j][:], k_ps2[j][:],
                                                 IDENT, bias=bk_sb[:])

            # --- phase 2 (rest): remaining PSUM/SBUF pools open now
            # that the chm tiles are gone ---
            psA = stk.enter_context(
                tc.tile_pool(name="psA", bufs=4, space="PSUM"))
            scrp = stk.enter_context(tc.tile_pool(name="scr", bufs=2))
            rzp = stk.enter_context(tc.tile_pool(name="rzp", bufs=1))
            m1p = stk.enter_context(tc.tile_pool(name="m1p", bufs=1))
            op = stk.enter_context(tc.tile_pool(name="op", bufs=1))
            if True:
                # prologue: sub-block 0's first-half pairs were already
                # emitted under the chm scope; pick up their tree quarters
                tree_q(0, 0)
                tree_q(0, 1)
                for t in range(8, 16):
                    score_pair(0, t)
                    if t % 4 == 3:
                        tree_q(0, t // 4)
                tree_folds(0)
                for t in range(4):
                    score_pair(1, t)
                tree_q(1, 0)
                zc_recip(0)
                for t in range(4, 8):
                    score_pair(1, t)
                tree_q(1, 1)
                norm_one(0)
                for t in range(8, 16):
                    score_pair(1, t)
                    if t % 4 == 3:
                        tree_q(1, t // 4)

                # steady loop: weave a later sub-block's score pairs into
                # this sub-block's attend matmuls; the NEXT sub-block's Z
                # pipeline is hoisted to the FRONT of this attend (its tree
                # finished during the previous attend) so its normalize and
                # cast-DMAs complete long before its own attend starts.
                weave = {0: [(2, t) for t in range(16)],
                         1: [(3, t) for t in range(16)],
                         2: [], 3: []}
                for s in range(NS):
                    w = SBS[s][2]
                    m1_sb = m1p.tile([P, ck, 512], BF16, tag="m1",
                                     name=f"m1_{s}")
                    m_ps = [psA.tile([P, 512], F32, tag="aps",
                                     name=f"mps{s}_{ch}") for ch in range(ck)]

                    def next_z(s=s):
                        tree_folds(s + 1)
                        zc_recip(s + 1)
                        norm_one(s + 1)

                    def next_z_tail(s=s):
                        # s=3's tree inputs arrive only at the very end of
                        # the exp backbone; run its folds as soon as they're
                        # ready and give the whole chain scheduler priority
                        # over the parked o_adds so the DVE wait-queue never
                        # blocks it.
                        with tc.high_priority(offset=200):
                            tree_folds(s + 1)
                            zc_recip(s + 1)
                            norm_one(s + 1)

                    mid = next_z if s + 1 < NS - 1 else None
                    end = next_z_tail if s + 1 == NS - 1 else None
                    attend_m1(s, m_ps, weave[s], mid_cb=mid, end_cb=end)
                    for ch in range(ck):
                        # s=0's copies would interrupt the ACT exp backbone
                        # (between exps(2) and exps(3)); DVE has slack there.
                        if s == 0:
                            nc.vector.tensor_copy(m1_sb[:, ch, 0:w],
                                                  m_ps[ch][:, 0:w])
                        else:
                            nc.scalar.copy(m1_sb[:, ch, 0:w], m_ps[ch][:, 0:w])
                    att2_store(s, m1_sb)
            stk.close()

    nc.compile()
    return nc


_NC_CACHE = {}


def _get_nc():
    if "nc" not in _NC_CACHE:
        _NC_CACHE["nc"] = build_program()
    return _NC_CACHE["nc"]


def _bf16(a):
    import ml_dtypes
    return np.ascontiguousarray(a.astype(ml_dtypes.bfloat16))


def _fp8(a):
    import ml_dtypes
    return np.ascontiguousarray(a.astype(ml_dtypes.float8_e4m3))


def make_in_maps(rgb_features, chm_features, Wq, bq, Wk, bk, Wv, bv, gamma):
    rgb_features = np.asarray(rgb_features, dtype=np.float32)
    chm_features = np.asarray(chm_features, dtype=np.float32)
    Wq = np.asarray(Wq, dtype=np.float32)
    Wk = np.asarray(Wk, dtype=np.float32)
    Wv = np.asarray(Wv, dtype=np.float32)
    bq = np.asarray(bq, dtype=np.float32)
    bk = np.asarray(bk, dtype=np.float32)
    bv = np.asarray(bv, dtype=np.float32)
    g = float(np.asarray(gamma).reshape(-1)[0])

    wq2 = _bf16(np.concatenate([Wq.T, Wq.T], axis=1))
    wk2 = _bf16(np.concatenate([Wk.T, Wk.T], axis=1))
    wv2 = _bf16((g * Wv).T)
    # softmax rows sum to 1 per (hw, x); summing over the 64 x's makes the
    # bias term contribute exactly 64*gamma*bv[c] to every output pixel.
    rgb_adj = rgb_features + (64.0 * g * bv)[None, :, None, None]
    bq2 = np.ascontiguousarray(np.concatenate([bq, bq]).reshape(2 * CQK, 1))
    bk2 = np.ascontiguousarray(np.concatenate([bk, bk]).reshape(2 * CQK, 1))
    # comb[p, i] = (p % 64 == i % 64): folds the two partition y-halves of
    # the tree result and replicates across both halves in one matmul.
    comb = _bf16(np.tile(np.eye(CQK, dtype=np.float32), (2, 2)))

    in_maps = []
    for core in range(N_CORES):
        b, half = divmod(core, 2)
        rgb_c = _bf16(
            rgb_adj[b].reshape(C, HW)[:, half * HWC:(half + 1) * HWC])
        # chm with xy permuted to (y, x) order: col' = y*64 + x.
        chm_yx = (chm_features[b].reshape(C, H, W)
                  .transpose(0, 2, 1).reshape(C, XY))
        chm_c = _bf16(chm_yx)
        # chm^T in fp8, laid out [q, m, ch, p] with xy' = m*128 + q and
        # cin = ch*128 + p, ready for the DoubleRow stationary APs.
        chmT8_c = _fp8(chm_yx.reshape(4, 128, 32, 128)
                       .transpose(3, 2, 0, 1).reshape(P, XY * 4))
        in_maps.append({
            "rgb": rgb_c, "chm": chm_c, "chmT8": chmT8_c,
            "wqk": np.ascontiguousarray(
                np.concatenate([wq2, wk2], axis=1)), "wv": wv2,
            "bs": np.ascontiguousarray(
                np.concatenate([bq2, bk2], axis=1)), "comb": comb,
        })
    return in_maps


def assemble(results):
    fused = np.empty((B, C, H, W), dtype=np.float32)
    fused2 = fused.reshape(B, C, HW)
    for core in range(N_CORES):
        b, half = divmod(core, 2)
        fused2[b, :, half * HWC:(half + 1) * HWC] = results[core]["out"]
    return fused


def kernel(rgb_features, chm_features, Wq, bq, Wk, bk, Wv, bv, gamma):
    nc = _get_nc()
    in_maps = make_in_maps(rgb_features, chm_features, Wq, bq, Wk, bk, Wv, bv,
                           gamma)
    res = run_bass_kernel_spmd(nc, in_maps, core_ids=list(range(N_CORES)))
    return assemble(res.results)
